# revision 24
# baseline (speedup 1.0000x reference)
"""Batched Viterbi (max-sum) CRF decode on 8 Trainium2 NeuronCores.

Problem: input_x [1024, 256, 128] f32, weights [26, 128], transition [26, 26].
emissions e = x @ W^T; forward scan delta_t[k] = max_j(delta_{t-1}[j] + T[j,k]) + e_t[k];
backtrack the argmax path. Output: labels [1024, 256] int32.

Sharding: pure data parallel - batch 1024 split over 8 cores (128 rows/core, one
batch row per SBUF partition). Weights/transition replicated.

Forward scan: ONE tensor_tensor_scan per step over a 676-wide (k-outer,
j-inner) T table computes all 26 windowed max-plus reductions:
  state'_j = max(state'_{j-1} + ddiff_j, T[j,k]),  ddiff_j = delta_{j-1}-delta_j
with -BIG in ddiff slot 0 resetting each window; window ends hold
max_j(delta_j + T[j,k]) - delta_25. Global offsets cancel in every argmax, so
the recursion tracks pseudo-deltas pd_t = scanout_ends + e_t (no offset fixup).

Backtrack: per-step recompute prev = argmax_j(pd_t[j] + T[j,y]), latency-
minimized: the one-hot of y is 32x32-block transposed on DVE (stream
transpose), one PE matmul against a block-diagonal T^T selects T[:, y] per
row, a second stream transpose brings it back, then a 26-wide add + max8 +
is_equal yields the next one-hot. Labels are extracted from the one-hot
history in bulk chunks interleaved into the (latency-bound) loop.

This container's walrus accepts at most one semaphore wait per instruction,
while Tile emits several on the kernel-tail drain - patched below by splitting
waits onto chained drains / NoOps. GPSIMD software ops don't codegen here.
"""

import functools

import numpy as np

B, S, D, K = 1024, 256, 128, 26
NCORES = 8
BSH = B // NCORES  # 128 batch rows per core == SBUF partition count
KK = K * K  # 676
TC = 64  # time steps per x-staging chunk
NEG = -1.0e30


def _patch_tile_drain():
    """Split the kernel-tail drain's sem waits across chained drain
    instructions (this walrus allows one wait per instruction)."""
    import concourse.mybir as mybir
    from concourse.tile import TileContext
    from concourse.vector_clock import ScopedClock

    if getattr(TileContext, "_drain_split_patched", False):
        return

    def patched(self, tick_clock, wait_clock):
        nc = self.nc
        drain_inst = nc.sync.drain()
        wait_clock.add_sem_waits(
            drain_inst.ins, ScopedClock({None: tick_clock.global_clock})
        )
        raw = drain_inst.ins
        si = raw.sync_info
        waits = list(si.on_wait)
        if len(waits) > 1:
            raw.sync_info = mybir.SyncInfo(
                on_wait=waits[:1], on_update=list(si.on_update)
            )
            for w in waits[1:]:
                extra = nc.sync.drain()
                extra.ins.sync_info = mybir.SyncInfo(on_wait=[w], on_update=[])
        nc.all_engine_barrier()
        popped = nc._tile_sem_poison_stack.pop()
        assert popped is self._sem_poison
        nc.clear_and_free_semaphores(list(self.sems.allocated().values()))
        nc.all_engine_barrier()

    TileContext._drain_and_barrier = patched
    TileContext._drain_split_patched = True


def _split_multiwaits(nc):
    """Hoist extra sem waits (>1 per instruction) onto preceding NoOps."""
    import concourse.mybir as mybir

    cnt = 0
    for f in nc.m.functions:
        for bb in f.blocks:
            insts = bb.instructions
            new_list = []
            changed = False
            for inst in insts:
                si = getattr(inst, "sync_info", None)
                waits = list(si.on_wait) if si is not None else []
                if len(waits) > 1:
                    for w in waits[:-1]:
                        nop = mybir.InstNoOp(name=f"mwsplit-{cnt}", ins=[], outs=[])
                        cnt += 1
                        nop.engine = inst.engine
                        nop.sync_info = mybir.SyncInfo(on_wait=[w], on_update=[])
                        new_list.append(nop)
                    inst.sync_info = mybir.SyncInfo(
                        on_wait=[waits[-1]], on_update=list(si.on_update)
                    )
                    changed = True
                new_list.append(inst)
            if changed:
                insts[:] = new_list
    return cnt


def _ttss(nc, out, data0, data1, initial, op0, op1):
    """tensor_tensor_scan accepting a multi-free-dim (broadcast) data0 view.

    Mirrors BassVectorEngine.tensor_tensor_scan minus the 2D-only assert: the
    scan runs in flat AP iteration order, which for our [p, k(bcast), j] view
    is exactly the window-repeated ddiff sequence (verified on HW).
    """
    import concourse.mybir as mybir

    eng = nc.vector
    return eng.add_instruction(
        mybir.InstTensorScalarPtr(
            name=nc.get_next_instruction_name(),
            is_tensor_tensor_scan=True,
            is_scalar_tensor_tensor=True,
            op0=op0,
            op1=op1,
            ins=[
                eng.lower_ap(data0),
                eng.lower_ap_or_imm(initial),
                eng.lower_ap(data1),
            ],
            outs=[eng.lower_ap(out)],
        )
    )


@functools.cache
def _build(build_stage="full"):
    import concourse.bass as bass
    import concourse.mybir as mybir
    from concourse.tile import TileContext

    _patch_tile_drain()

    F32 = mybir.dt.float32
    OP = mybir.AluOpType
    AX = mybir.AxisListType

    nc = bass.Bass()
    x = nc.dram_tensor("x", [BSH, S, D], F32, kind="ExternalInput")
    w = nc.dram_tensor("w", [K, D], F32, kind="ExternalInput")
    t_in = nc.dram_tensor("t", [K, K], F32, kind="ExternalInput")
    y_out = nc.dram_tensor("y", [BSH, S], mybir.dt.int32, kind="ExternalOutput")

    ident_c = nc.inline_tensor(np.eye(BSH, dtype=np.float32), name="identc")
    iota_c = nc.inline_tensor(
        np.tile(np.arange(K, dtype=np.float32), (BSH, 1)), name="iotac"
    )
    ones_c = nc.inline_tensor(np.ones((1, BSH), dtype=np.float32), name="onesc")

    with (
        TileContext(nc) as tc,
        tc.tile_pool(name="const", bufs=1) as cpool,
        tc.tile_pool(name="hist", bufs=1) as hpool,
        tc.tile_pool(name="stage", bufs=2) as spool,
        tc.tile_pool(name="work", bufs=3) as wpool,
        tc.tile_pool(name="scan", bufs=3) as scpool,
        tc.tile_pool(name="bt", bufs=4) as btpool,
        tc.tile_pool(name="psum_e", bufs=2, space="PSUM") as ppool,
        tc.tile_pool(name="psum_xt", bufs=2, space="PSUM") as ppool_xt,
        tc.tile_pool(name="psum_bt", bufs=2, space="PSUM") as ppool_bt,
    ):
        # ---------------- constants ----------------
        ident = cpool.tile([BSH, BSH], F32)
        nc.sync.dma_start(out=ident[:], in_=ident_c[:])
        iota_f = cpool.tile([BSH, K], F32)
        nc.sync.dma_start(out=iota_f[:], in_=iota_c[:])
        ones1 = cpool.tile([1, BSH], F32)
        nc.sync.dma_start(out=ones1[:], in_=ones_c[:])

        wt = cpool.tile([D, K], F32)  # W^T [d, k]
        nc.sync.dma_start(out=wt[:], in_=w[:].rearrange("k d -> d k"))

        # T row-major flat on one partition (1-descriptor DMA), replicated to
        # all partitions via PE ones-matmul; the TTSS reads it through a
        # transposed (k-outer, j-inner) strided view.
        tt0 = cpool.tile([1, KK], F32)
        nc.sync.dma_start(
            out=tt0[:],
            in_=t_in[:].rearrange("j k -> (j k)").rearrange("(o f) -> o f", o=1),
        )
        tord = cpool.tile([BSH, KK], F32)
        half = KK // 2  # 338: fits one PSUM bank
        for h in range(2):
            rep_ps = ppool_xt.tile([BSH, half], F32, tag="xt")
            nc.tensor.matmul(
                rep_ps[:],
                ones1[:],
                tt0[:, h * half : (h + 1) * half],
                start=True,
                stop=True,
            )
            nc.vector.tensor_copy(tord[:, h * half : (h + 1) * half], rep_ps[:])

        # T^T [k, j] for the backtrack column-select matmul, and a 4-block
        # diagonal [128, 128] version of it matching the 32-row blocks that
        # DVE stream_transpose produces: bd[32q+k, 32q+j] = T[j, k].
        t_sb = cpool.tile([K, K], F32)
        nc.sync.dma_start(out=t_sb[:], in_=t_in[:])
        ttr_ps = ppool_xt.tile([K, K], F32, tag="xt")
        nc.tensor.transpose(ttr_ps[:], t_sb[:], ident[:K, :K])
        tt_T = cpool.tile([K, K], F32)
        nc.scalar.copy(out=tt_T[:], in_=ttr_ps[:])
        bd = cpool.tile([BSH, BSH], F32)
        nc.vector.memset(bd[:], 0.0)

        # pseudo-delta history [b, t*K + k]; emissions staged to SBUF by ACT
        hist = hpool.tile([BSH, S * K], F32)
        e_hist = hpool.tile([BSH, S * K], F32)
        # ddiff[., 0] = -BIG resets each scan window; slots 1..25 rewritten
        # every step with adjacent pseudo-delta differences
        ddiff = hpool.tile([BSH, K], F32)
        nc.vector.memset(ddiff[:, 0:1], NEG)

        # ---------------- emissions (PE) ----------------
        pending = None  # issue each e-matmul one step late so the ACT
        # PSUM->SBUF copy overlaps the next transpose
        chunks = [8, 56] + [TC] * ((S - TC) // TC)
        assert sum(chunks) == S
        t0 = 0
        for clen in chunks:
            stage = spool.tile([BSH, TC * D], F32, tag="stage")
            nc.sync.dma_start(
                out=stage[:, : clen * D],
                in_=x[:, t0 : t0 + clen, :].rearrange("b t d -> b (t d)"),
            )
            for tl in range(clen):
                t = t0 + tl
                xt_ps = ppool_xt.tile([D, BSH], F32, tag="xt")
                nc.tensor.transpose(xt_ps[:], stage[:, tl * D : (tl + 1) * D], ident[:])
                xt_sb = wpool.tile([D, BSH], F32, tag="xts")
                nc.scalar.copy(out=xt_sb[:], in_=xt_ps[:])
                if pending is not None:
                    pt, psb = pending
                    e_ps = ppool.tile([BSH, K], F32, tag="e")
                    nc.tensor.matmul(e_ps[:], psb[:], wt[:], start=True, stop=True)
                    nc.scalar.copy(out=e_hist[:, pt * K : (pt + 1) * K], in_=e_ps[:])
                pending = (t, xt_sb)
            t0 += clen
        pt, psb = pending
        e_ps = ppool.tile([BSH, K], F32, tag="e")
        nc.tensor.matmul(e_ps[:], psb[:], wt[:], start=True, stop=True)
        nc.scalar.copy(out=e_hist[:, pt * K : (pt + 1) * K], in_=e_ps[:])

        # block-diagonal T^T loaded late so these strided DMAs queue behind
        # the emission-critical ones (bd is first used ~300us in)
        for q in range(4):
            sl = slice(32 * q, 32 * q + K)
            nc.sync.dma_start(out=bd[sl, sl], in_=t_in[:].rearrange("j k -> k j"))

        # ---------------- forward scan (DVE) ----------------
        # t = 0: pseudo-delta = e_0
        nc.vector.tensor_copy(hist[:, 0:K], e_hist[:, 0:K])
        nc.vector.tensor_tensor(
            out=ddiff[:, 1:K],
            in0=hist[:, 0 : K - 1],
            in1=hist[:, 1:K],
            op=OP.subtract,
        )
        tord_kj = tord[:].rearrange("p (j k) -> p k j", k=K)
        n_fwd = S if build_stage in ("full", "fwd") else 2
        for t in range(1, n_fwd):
            scanout = scpool.tile([BSH, KK], F32, tag="scan")
            d0 = (
                ddiff[:]
                .rearrange("p (o j) -> p o j", o=1)
                .to_broadcast([BSH, K, K])
            )
            _ttss(nc, scanout[:], d0, tord_kj, NEG, OP.add, OP.max)
            hs = hist[:, t * K : (t + 1) * K]
            nc.vector.tensor_tensor(
                out=hs,
                in0=scanout[:, K - 1 : KK : K],
                in1=e_hist[:, t * K : (t + 1) * K],
                op=OP.add,
            )
            if t < S - 1:
                nc.vector.tensor_tensor(
                    out=ddiff[:, 1:K],
                    in0=hist[:, t * K : (t + 1) * K - 1],
                    in1=hist[:, t * K + 1 : (t + 1) * K],
                    op=OP.subtract,
                )

        # ---------------- backtrack ----------------
        # Single serial chain, latency-minimized: the per-step state is the
        # one-hot of y_t written into a 32-padded history. Each step:
        #   DVE stream_transpose (one-hot -> 32x32-blocked ohT) ->
        #   PE matmul with the block-diagonal T^T (selects T[:, y] per row,
        #   blocked) -> DVE stream_transpose back -> 26-wide add of hist ->
        #   max8 -> is_equal (next one-hot from the max value).
        # No ACT and no PE-transpose round trip; labels extracted from the
        # one-hot history in one bulk pass at the end.
        ohh = hpool.tile([BSH, S * 32], F32)
        nc.vector.memset(ohh[:], 0.0)

        def bt_dve(t, src):
            """max8 + one-hot of argmax into ohh[:, 32t:32t+26]."""
            max8 = btpool.tile([BSH, 8], F32, tag="max8")
            nc.vector.max(out=max8[:], in_=src)
            nc.vector.tensor_tensor(
                ohh[:, 32 * t : 32 * t + K],
                src,
                max8[:, 0:1].to_broadcast([BSH, K]),
                op=OP.is_equal,
            )

        # chunked label extraction, interleaved into the (latency-bound)
        # backtrack loop as soon as each t-range of one-hots is complete:
        # y_t = max_k(onehot[t,k] * k), mult in place
        y_f = hpool.tile([BSH, S], F32)
        iota3 = lambda n: (  # noqa: E731
            iota_f[:].rearrange("p (o k) -> p o k", o=1).to_broadcast([BSH, n, K])
        )

        def extract(t0, t1):
            n = t1 - t0
            oh3 = ohh[:, 32 * t0 : 32 * t1].rearrange("p (t w) -> p t w", w=32)[
                :, :, 0:K
            ]
            nc.vector.tensor_tensor(oh3, oh3, iota3(n), op=OP.mult)
            nc.vector.reduce_max(y_f[:, t0:t1], oh3, axis=AX.X)

        EC = 64  # extract-chunk length
        bt_dve(S - 1, hist[:, (S - 1) * K : S * K])
        bt_stop = 0 if build_stage == "full" else S - 2
        for t in range(S - 2, bt_stop - 1, -1):
            ohTb = btpool.tile([BSH, 32], F32, tag="ohTb")
            nc.vector.transpose(out=ohTb[:], in_=ohh[:, 32 * (t + 1) : 32 * (t + 2)])
            tcolT_ps = ppool_bt.tile([BSH, 32], F32, tag="bt")
            nc.tensor.matmul(tcolT_ps[:], bd[:], ohTb[:], start=True, stop=True)
            tcb = btpool.tile([BSH, 32], F32, tag="tcb")
            nc.vector.transpose(out=tcb[:], in_=tcolT_ps[:])
            tmp2 = btpool.tile([BSH, K], F32, tag="tmp2")
            nc.vector.tensor_tensor(
                tmp2[:], tcb[:, 0:K], hist[:, t * K : (t + 1) * K], op=OP.add
            )
            bt_dve(t, tmp2[:])
            if build_stage == "full" and (t + 2) % EC == 0 and t + 2 < S:
                extract(t + 2, t + 2 + EC)
        if build_stage == "full":
            extract(0, EC)
        else:
            extract(bt_stop, S)

        y_i = hpool.tile([BSH, S], mybir.dt.int32)
        nc.vector.tensor_copy(y_i[:], y_f[:])
        nc.sync.dma_start(out=y_out[:], in_=y_i[:])

    n = _split_multiwaits(nc)
    if n:
        import logging

        logging.getLogger(__name__).info("split %d multi-wait instructions", n)
    return nc


def run(input_x, weights, transition, **spmd_kwargs):
    from concourse.bass_utils import run_bass_kernel_spmd

    nc = _build()
    input_x = np.ascontiguousarray(np.asarray(input_x, dtype=np.float32))
    weights = np.ascontiguousarray(np.asarray(weights, dtype=np.float32))
    transition = np.ascontiguousarray(np.asarray(transition, dtype=np.float32))
    in_maps = [
        {
            "x": input_x[i * BSH : (i + 1) * BSH],
            "w": weights,
            "t": transition,
        }
        for i in range(NCORES)
    ]
    res = run_bass_kernel_spmd(nc, in_maps, core_ids=list(range(NCORES)), **spmd_kwargs)
    out = np.concatenate([r["y"] for r in res.results], axis=0).astype(np.int32)
    return out, res


def kernel(input_x, weights, transition):
    out, _ = run(input_x, weights, transition)
    return out
